# revision 2
# baseline (speedup 1.0000x reference)
"""Trainium2 Bass kernel for nn_ConditionedConvolution2D.

Reference computation:
    A  = P @ dense_w                      # [B, 3*3*C*C_OUT] per-sample conv kernels
    Wk = A.reshape(B, 3, 3, C, C_OUT)
    Y[b] = conv2d(X[b], Wk[b])            # SAME padding, stride 1, NHWC

Strategy (pure data parallel, 4 samples per core on 8 cores):
  - Host pre-lays X as a bf16 "shifted triple" X_trip[b, dw*32+ci, hp, wp] =
    X_padded[b, ci, hp, wp+dw] so the device can read, for every padded row hp,
    a ready-made im2col stationary lhsT [96=(dw,ci), 128=w] with a single AP.
  - Device computes the hypernetwork (per-sample kernels) with 96 small
    matmuls from a host-permuted dense_w so the weights land directly in
    [(dw,ci), (dh,co)] streaming layout, then casts to bf16.
  - Conv: for each padded row hp, 3 matmuls (one per dh) accumulate
    out[w, co] rows r = hp-dh in rotating PSUM tiles (fp32 accumulation).
  - Completed rows are copied PSUM->SBUF (DVE/ACT alternating) and DMA'd out
    in 16-row blocks to a [b, w, h*co] DRAM layout (contiguous 2KB runs);
    host transposes back to NHWC.
"""

import os
import sys

sys.path.insert(0, "/opt/trn_rl_repo")

import numpy as np
import ml_dtypes

import concourse.bacc as bacc
import concourse.mybir as mybir
import concourse.tile as tile
from concourse.bass_utils import run_bass_kernel_spmd

B, H, W, C = 32, 128, 128, 32
P_DIM = 128
KH = KW = 3
C_OUT = 32
N_CORES = 8
BPC = B // N_CORES          # samples per core
H2 = H + 2                  # padded rows
W2 = W + 4                  # padded row pitch (2 pad cols + 2 alignment)
QK = KW * C                 # 96 contraction size (dw, ci)
G = KH * C_OUT              # 96 weight-stream columns per sample (dh, co)

USE_LDW = os.environ.get("CONV_USE_LDW", "1") == "1"
ROWS_PER_OSB = 16

_NC_CACHE = {}


def _build_nc():
    f32 = mybir.dt.float32
    bf16 = mybir.dt.bfloat16
    nc = bacc.Bacc("TRN2", target_bir_lowering=False, debug=False,
                   num_devices=N_CORES)
    x_trip = nc.dram_tensor("x_trip", [BPC, QK, H2 * W2], bf16,
                            kind="ExternalInput")
    p_t = nc.dram_tensor("p_t", [P_DIM, BPC], bf16, kind="ExternalInput")
    dw_t = nc.dram_tensor("dw_t", [P_DIM, KH * KW * C * C_OUT], bf16,
                          kind="ExternalInput")
    y = nc.dram_tensor("y", [BPC, W, H * C_OUT], f32, kind="ExternalOutput")

    with tile.TileContext(nc) as tc:
        with tc.tile_pool(name="const", bufs=1) as cpool, \
             tc.tile_pool(name="wsb", bufs=1) as wsb_pool, \
             tc.tile_pool(name="slab", bufs=2) as slab_pool, \
             tc.tile_pool(name="osb", bufs=3) as osb_pool:

            # ---- Phase 0: hypernetwork  Wk = P @ dense_w (permuted) ----
            p_sb = cpool.tile([P_DIM, BPC], bf16, name="p_sb", tag="p_sb")
            nc.sync.dma_start(out=p_sb[:], in_=p_t[:])
            dwsb = cpool.tile([P_DIM, KH * KW * C * C_OUT], bf16, name="dwsb", tag="dwsb")
            nc.sync.dma_start(out=dwsb[:], in_=dw_t[:])

            # w_sb[q=(dw,ci), b*G + dh*C_OUT + co] (bf16 stream operand)
            w_sb = wsb_pool.tile([QK, BPC * G], bf16, name="w_sb", tag="w_sb")

            with tc.tile_pool(name="wps", bufs=2, space="PSUM") as wps_pool:
                for half in range(2):
                    wps = wps_pool.tile([QK, 48 * BPC], f32, name="wps", tag="wps")
                    for gg in range(48):
                        g = half * 48 + gg      # g = dh*C_OUT + co
                        nc.tensor.matmul(
                            out=wps[:, gg * BPC:(gg + 1) * BPC],
                            lhsT=dwsb[:, g * QK:(g + 1) * QK],
                            rhs=p_sb[:],
                            start=True, stop=True,
                        )
                    # permute (g, b) -> (b, g) while casting f32 -> bf16
                    src = wps[:].rearrange("p (g b) -> p g b", b=BPC)
                    dst = w_sb[:].rearrange("p (b g) -> p g b", g=G)[
                        :, half * 48:(half + 1) * 48, :]
                    nc.vector.tensor_copy(out=dst, in_=src)

            # ---- Phase 1: per-sample conv ----
            with tc.tile_pool(name="acc", bufs=6, space="PSUM") as acc_pool:
                for b in range(BPC):
                    slab = slab_pool.tile([QK, H2 * W2], bf16, name="slab", tag="slab")
                    nc.sync.dma_start(out=slab[:], in_=x_trip[b])

                    psum_rot = {}
                    osb = None
                    for hp in range(H2):
                        lhsT = slab[:, hp * W2: hp * W2 + W]
                        if USE_LDW:
                            nc.tensor.ldweights(lhsT)
                        for dh in range(KH):
                            r = hp - dh
                            if not (0 <= r < H):
                                continue
                            if dh == 0:
                                psum_rot[r] = acc_pool.tile([W, C_OUT], f32,
                                                            name="acc", tag="acc")
                            nc.tensor.matmul(
                                out=psum_rot[r][:],
                                lhsT=lhsT,
                                rhs=w_sb[:, b * G + dh * C_OUT:
                                         b * G + (dh + 1) * C_OUT],
                                start=(dh == 0), stop=(dh == KH - 1),
                            )
                        r_done = hp - (KH - 1)
                        if 0 <= r_done < H:
                            k = r_done % ROWS_PER_OSB
                            if k == 0:
                                osb = osb_pool.tile([W, ROWS_PER_OSB * C_OUT],
                                                    f32, name="osb", tag="osb")
                            dst = osb[:, k * C_OUT:(k + 1) * C_OUT]
                            if r_done % 2 == 0:
                                nc.vector.tensor_copy(out=dst,
                                                      in_=psum_rot[r_done][:])
                            else:
                                nc.scalar.copy(out=dst, in_=psum_rot[r_done][:])
                            del psum_rot[r_done]
                            if k == ROWS_PER_OSB - 1:
                                r0 = r_done - (ROWS_PER_OSB - 1)
                                nc.sync.dma_start(
                                    out=y[b][:, r0 * C_OUT:
                                             (r_done + 1) * C_OUT],
                                    in_=osb[:],
                                )
    nc.finalize()
    return nc


def _get_nc():
    if "nc" not in _NC_CACHE:
        _NC_CACHE["nc"] = _build_nc()
    return _NC_CACHE["nc"]


def _prep_inputs(X, P, dense_w):
    bf16 = ml_dtypes.bfloat16
    Xb = np.ascontiguousarray(X.transpose(0, 3, 1, 2)).astype(bf16)  # [B,C,H,W]
    X_trip = np.zeros((B, QK, H2, W2), dtype=bf16)
    for dw in range(KW):
        lo = max(0, 1 - dw)          # first valid wp
        hi = W - dw                  # last valid wp (inclusive)
        src_lo = lo + dw - 1         # = max(dw-1, 0)... for dw=0: 0
        X_trip[:, dw * C:(dw + 1) * C, 1:H + 1, lo:hi + 1] = \
            Xb[:, :, :, src_lo:W]
    X_trip = X_trip.reshape(B, QK, H2 * W2)

    # dense_w columns j = ((dh*3+dw)*C+ci)*C_OUT+co  ->  (dh, co, dw, ci)
    dwp = np.ascontiguousarray(
        dense_w.reshape(P_DIM, KH, KW, C, C_OUT)
        .transpose(0, 1, 4, 2, 3)
        .reshape(P_DIM, -1)
    ).astype(bf16)

    in_maps = []
    for c in range(N_CORES):
        sl = slice(c * BPC, (c + 1) * BPC)
        in_maps.append({
            "x_trip": np.ascontiguousarray(X_trip[sl]),
            "p_t": np.ascontiguousarray(P[sl].T).astype(bf16),
            "dw_t": dwp,
        })
    return in_maps


def _run(X, P, dense_w, **spmd_kwargs):
    nc = _get_nc()
    in_maps = _prep_inputs(X, P, dense_w)
    res = run_bass_kernel_spmd(nc, in_maps, core_ids=list(range(N_CORES)),
                               **spmd_kwargs)
    outs = []
    for c in range(N_CORES):
        yv = res.results[c]["y"].reshape(BPC, W, H, C_OUT)
        outs.append(yv.transpose(0, 2, 1, 3))        # -> [b, h, w, co]
    Y = np.ascontiguousarray(np.concatenate(outs, axis=0), dtype=np.float32)
    return Y, res


def kernel(X, P, dense_w):
    Y, _ = _run(np.asarray(X), np.asarray(P), np.asarray(dense_w))
    return Y
